# revision 2
# baseline (speedup 1.0000x reference)
"""Trainium2 Bass kernel for nn_EnhancedSocialRecommender (gnn_message_passing).

Self-contained: takes the FULL (unsharded) inputs, shards internally across 8
NeuronCores (1D row-range partition of users/items; edge lists sorted by
destination tile per core), runs one SPMD Bass/Tile program with AllGather /
AllReduce collectives between layers, and returns the full [4096] f32 scores.

Segment-sums are computed per 128-row destination tile:
  per 128-edge block: indirect row gather (GPSIMD DMA) + one-hot build
  (DVE tensor_scalar is_equal*w) + TensorE matmul accumulation in PSUM.
Layer fusion: the u-direction pass gathers interleaved [it0 | i1] row pairs
(512B descriptors cost the same as 256B) and computes u1 and u2 in one sweep.
Passes whose outputs are only read at batch rows (i2, s1, s2) are pruned to
the exact backward-reachable destination sets.
"""
import sys
sys.path.insert(0, '/opt/trn_rl_repo')

import numpy as np
import ml_dtypes
from contextlib import ExitStack
from dataclasses import dataclass, field

import concourse.bass as bass
import concourse.tile as tile
import concourse.mybir as mybir
from concourse import bacc

P = 128
BF = mybir.dt.bfloat16
F32 = mybir.dt.float32
I32 = mybir.dt.int32
ALU = mybir.AluOpType
AF = mybir.ActivationFunctionType
AX = mybir.AxisListType
BF16 = ml_dtypes.bfloat16

U_FULL, I_FULL, D_EMB = 100000, 150000, 64


@dataclass
class Cfg:
    NCORE: int
    D: int
    U_PER: int
    I_PER: int
    B: int
    nw: tuple
    res: float = 0.1
    eps: float = 1e-5
    slope: float = 0.01
    nblk_i1: list = field(default_factory=list)
    nblk_uf: list = field(default_factory=list)
    nblk_i2: list = field(default_factory=list)
    nblk_s1: list = field(default_factory=list)
    nblk_s2: list = field(default_factory=list)

    @property
    def UT(self): return self.U_PER // P
    @property
    def IT(self): return self.I_PER // P
    @property
    def BT(self): return self.B // P
    @property
    def U_PAD(self): return self.U_PER * self.NCORE
    @property
    def I_PAD(self): return self.I_PER * self.NCORE


# ============================ program builder ============================

def build(cfg: Cfg):
    D = cfg.D
    nc = bacc.Bacc("TRN2", target_bir_lowering=False, debug=False,
                   num_devices=cfg.NCORE)

    def inp(name, shape, dt):
        return nc.dram_tensor(name, shape, dt, kind="ExternalInput")

    t_ue = inp("ue", [cfg.U_PAD, D], BF)
    t_ue_own = inp("ue_own", [cfg.U_PER, D], BF)
    t_ie_own = inp("ie_own", [cfg.I_PER, D], BF)

    eb = {}
    for nm, nblk in (("i1", cfg.nblk_i1), ("uf", cfg.nblk_uf), ("i2", cfg.nblk_i2),
                     ("s1", cfg.nblk_s1), ("s2", cfg.nblk_s2)):
        NB = max(1, int(np.sum(nblk)))
        eb[nm] = dict(
            src=inp(f"eb_{nm}_src", [P, NB], I32),
            do=inp(f"eb_{nm}_do", [P, NB], F32),
            w=inp(f"eb_{nm}_w", [P, NB], F32),
            NB=NB, nblk=nblk)

    t_iota = inp("iota", [P, P], BF)
    t_ident = inp("ident", [P, P], BF)
    t_identf = inp("identf", [P, P], F32)
    t_wsoc0 = inp("wsoc0", [D, D], BF)
    t_wsoc1 = inp("wsoc1", [D, D], BF)
    t_how = inp("how", [D, D], BF)
    t_mpw = inp("mpw", [2 * D, D], BF)
    t_hog = inp("hog", [P, D], F32)
    t_hob = inp("hob", [P, D], F32)
    t_hobeta = inp("hobeta", [P, D], F32)
    t_mpg = inp("mpg", [P, D], F32)
    t_mpb = inp("mpb", [P, D], F32)
    t_mpbeta = inp("mpbeta", [P, D], F32)
    t_pu_g = inp("pu_g", [P, cfg.BT], I32)
    t_pu_l = inp("pu_l", [P, cfg.BT], I32)
    t_pu_m = inp("pu_m", [P, cfg.BT], F32)
    t_pi_g = inp("pi_g", [P, cfg.BT], I32)
    t_pi_l = inp("pi_l", [P, cfg.BT], I32)
    t_pi_m = inp("pi_m", [P, cfg.BT], F32)

    d_it01_sh = nc.dram_tensor("it01_sh", [cfg.I_PER, 2 * D], BF)
    d_it01 = nc.dram_tensor("it01", [cfg.I_PAD, 2 * D], BF, addr_space="Shared")
    d_u1_sh = nc.dram_tensor("u1_sh", [cfg.U_PER, D], BF)
    d_u1 = nc.dram_tensor("u1f", [cfg.U_PAD, D], BF, addr_space="Shared")
    d_u2_sh = nc.dram_tensor("u2_sh", [cfg.U_PER, D], BF)
    d_i2_sh = nc.dram_tensor("i2_sh", [cfg.I_PER, D], BF)
    d_s1_sh = nc.dram_tensor("s1_sh", [cfg.U_PER, D], BF)
    d_s2_sh = nc.dram_tensor("s2_sh", [cfg.U_PER, D], BF)
    d_y0_sh = nc.dram_tensor("y0_sh", [cfg.U_PER, D], BF)
    d_y0 = nc.dram_tensor("y0f", [cfg.U_PAD, D], BF, addr_space="Shared")
    d_y1_sh = nc.dram_tensor("y1_sh", [cfg.U_PER, D], BF)
    d_y1 = nc.dram_tensor("y1f", [cfg.U_PAD, D], BF, addr_space="Shared")
    d_p_part = nc.dram_tensor("p_part", [cfg.B, D], F32)
    d_q_part = nc.dram_tensor("q_part", [cfg.B, D], F32)
    d_p = nc.dram_tensor("p_full", [cfg.B, D], F32, addr_space="Shared")
    d_q = nc.dram_tensor("q_full", [cfg.B, D], F32, addr_space="Shared")

    o_scores = nc.dram_tensor("scores", [cfg.B], F32, kind="ExternalOutput")

    groups = [list(range(cfg.NCORE))]

    with tile.TileContext(nc) as tc:
        ctx = ExitStack()
        with ctx:
            pw = ctx.enter_context(tc.tile_pool(name="pw", bufs=1))
            pmeta = ctx.enter_context(tc.tile_pool(name="pmeta", bufs=2))
            pg = ctx.enter_context(tc.tile_pool(name="pg", bufs=12))
            poh = ctx.enter_context(tc.tile_pool(name="poh", bufs=6))
            pe = ctx.enter_context(tc.tile_pool(name="pe", bufs=4))
            pper = ctx.enter_context(tc.tile_pool(name="pper", bufs=3))
            pps = ctx.enter_context(tc.tile_pool(name="pps", bufs=2, space="PSUM"))
            ppt = ctx.enter_context(tc.tile_pool(name="ppt", bufs=2, space="PSUM"))
            ppm = ctx.enter_context(tc.tile_pool(name="ppm", bufs=2, space="PSUM"))

            def load_pw(t_dram, shape, dt):
                t = pw.tile(shape, dt, tag=t_dram.name)
                nc.sync.dma_start(out=t[:], in_=t_dram[:, :] if len(shape) == 2 else t_dram[:])
                return t

            iota_t = load_pw(t_iota, [P, P], BF)
            ident_t = load_pw(t_ident, [P, P], BF)
            identf_t = load_pw(t_identf, [P, P], F32)
            wsoc0_t = load_pw(t_wsoc0, [D, D], BF)
            wsoc1_t = load_pw(t_wsoc1, [D, D], BF)
            how_t = load_pw(t_how, [D, D], BF)
            mpw_t = load_pw(t_mpw, [2 * D, D], BF)
            hog_t = load_pw(t_hog, [P, D], F32)
            hob_t = load_pw(t_hob, [P, D], F32)
            hobeta_t = load_pw(t_hobeta, [P, D], F32)
            mpg_t = load_pw(t_mpg, [P, D], F32)
            mpb_t = load_pw(t_mpb, [P, D], F32)
            mpbeta_t = load_pw(t_mpbeta, [P, D], F32)
            pu_g_t = load_pw(t_pu_g, [P, cfg.BT], I32)
            pu_l_t = load_pw(t_pu_l, [P, cfg.BT], I32)
            pu_m_t = load_pw(t_pu_m, [P, cfg.BT], F32)
            pi_g_t = load_pw(t_pi_g, [P, cfg.BT], I32)
            pi_l_t = load_pw(t_pi_l, [P, cfg.BT], I32)
            pi_m_t = load_pw(t_pi_m, [P, cfg.BT], F32)

            CHUNK = 2048

            eps_t = pw.tile([P, 1], F32, tag="eps")
            nc.gpsimd.memset(eps_t[:], cfg.eps)

            # zero-fill shard tensors that pruned passes may leave unwritten
            zt = pw.tile([P, 2048], BF, tag="zt")
            nc.gpsimd.memset(zt[:], 0.0)
            for dten, rows in ((d_i2_sh, cfg.I_PER), (d_s1_sh, cfg.U_PER),
                               (d_s2_sh, cfg.U_PER), (d_y1_sh, cfg.U_PER)):
                flat = dten[:, :].rearrange("r d -> (r d)")
                total = rows * D
                csz = P * 2048
                off = 0
                while off < total:
                    n = min(csz, total - off)
                    nper = n // P
                    nc.sync.dma_start(
                        out=flat[off:off + n].rearrange("(p f) -> p f", p=P),
                        in_=zt[:, :nper])
                    off += n

            def spmm_pass(nm, table_view, W, epilogue):
                e = eb[nm]
                nblk = e["nblk"]
                n_tiles = len(nblk)
                chunks = []
                t0, c0, acc = 0, 0, 0
                for t in range(n_tiles):
                    if acc + nblk[t] > CHUNK and acc > 0:
                        chunks.append((t0, t, c0, acc))
                        t0, c0, acc = t, c0 + acc, 0
                    acc += int(nblk[t])
                chunks.append((t0, n_tiles, c0, acc))
                for (ta, tb, col0, ncols) in chunks:
                    if ncols > 0:
                        src_t = pmeta.tile([P, ncols], I32, tag="src")
                        nc.sync.dma_start(out=src_t[:], in_=e["src"][:, col0:col0 + ncols])
                        do_t = pmeta.tile([P, ncols], F32, tag="do")
                        nc.sync.dma_start(out=do_t[:], in_=e["do"][:, col0:col0 + ncols])
                        w_t = pmeta.tile([P, ncols], F32, tag="w")
                        nc.sync.dma_start(out=w_t[:], in_=e["w"][:, col0:col0 + ncols])
                    col = 0
                    for t in range(ta, tb):
                        nb = int(nblk[t])
                        if nb == 0:
                            epilogue(t, None)
                            continue
                        psum = pps.tile([P, W], F32, tag="spmm")
                        for b in range(nb):
                            g = pg.tile([P, W], BF, tag="g")
                            nc.gpsimd.indirect_dma_start(
                                out=g[:], out_offset=None, in_=table_view,
                                in_offset=bass.IndirectOffsetOnAxis(
                                    ap=src_t[:, col:col + 1], axis=0))
                            oh = poh.tile([P, P], BF, tag="oh")
                            nc.vector.tensor_scalar(
                                out=oh[:], in0=iota_t[:],
                                scalar1=do_t[:, col:col + 1], scalar2=w_t[:, col:col + 1],
                                op0=ALU.is_equal, op1=ALU.mult)
                            nc.tensor.matmul(out=psum[:], lhsT=oh[:], rhs=g[:],
                                             start=(b == 0), stop=(b == nb - 1))
                            col += 1
                        epilogue(t, psum)

            def xw_rows(x_bf, w_t, K):
                psT = ppt.tile([K, P], BF, tag="tr")
                nc.tensor.transpose(out=psT[:], in_=x_bf[:], identity=ident_t[:])
                xT = pe.tile([K, P], BF, tag="xT")
                nc.vector.tensor_copy(out=xT[:], in_=psT[:])
                psY = ppm.tile([P, D], F32, tag="mm")
                nc.tensor.matmul(out=psY[:], lhsT=xT[:], rhs=w_t[:], start=True, stop=True)
                return psY

            def layernorm_lrelu(t0f, g_t, beta_t):
                s1 = pe.tile([P, 1], F32, tag="s1")
                nc.vector.reduce_sum(out=s1[:], in_=t0f[:], axis=AX.X)
                mean = pe.tile([P, 1], F32, tag="mean")
                nc.vector.tensor_scalar(out=mean[:], in0=s1[:], scalar1=1.0 / D,
                                        scalar2=None, op0=ALU.mult)
                xc = pe.tile([P, D], F32, tag="xc")
                nc.vector.tensor_scalar(out=xc[:], in0=t0f[:], scalar1=mean[:, 0:1],
                                        scalar2=None, op0=ALU.subtract)
                sq = pe.tile([P, D], F32, tag="sq")
                nc.vector.tensor_tensor(out=sq[:], in0=xc[:], in1=xc[:], op=ALU.mult)
                s2 = pe.tile([P, 1], F32, tag="s2")
                nc.vector.reduce_sum(out=s2[:], in_=sq[:], axis=AX.X)
                std = pe.tile([P, 1], F32, tag="std")
                nc.scalar.activation(out=std[:], in_=s2[:], func=AF.Sqrt,
                                     bias=eps_t[:, 0:1], scale=1.0 / D)
                rstd = pe.tile([P, 1], F32, tag="rstd")
                nc.vector.reciprocal(out=rstd[:], in_=std[:])
                xn = pe.tile([P, D], F32, tag="xn")
                nc.vector.tensor_scalar(out=xn[:], in0=xc[:], scalar1=rstd[:, 0:1],
                                        scalar2=None, op0=ALU.mult)
                yg = pe.tile([P, D], F32, tag="yg")
                nc.vector.tensor_tensor(out=yg[:], in0=xn[:], in1=g_t[:], op=ALU.mult)
                yb = pe.tile([P, D], F32, tag="yb")
                nc.vector.tensor_tensor(out=yb[:], in0=yg[:], in1=beta_t[:], op=ALU.add)
                out = pe.tile([P, D], F32, tag="lrl")
                nc.scalar.activation(out=out[:], in_=yb[:], func=AF.Lrelu,
                                     alpha=cfg.slope)
                return out

            # ---- P1: i1
            def i1_epilogue(t, psum):
                cat = pe.tile([P, 2 * D], BF, tag="cat")
                i0t = pper.tile([P, D], BF, tag="i0t")
                nc.sync.dma_start(out=i0t[:], in_=t_ie_own[t * P:(t + 1) * P, :])
                nc.vector.tensor_copy(out=cat[:, :D], in_=i0t[:])
                r = pe.tile([P, D], F32, tag="r01")
                nc.vector.tensor_scalar(out=r[:], in0=i0t[:], scalar1=cfg.res,
                                        scalar2=None, op0=ALU.mult)
                if psum is not None:
                    nc.vector.tensor_tensor(out=cat[:, D:], in0=psum[:], in1=r[:], op=ALU.add)
                else:
                    nc.vector.tensor_copy(out=cat[:, D:], in_=r[:])
                nc.sync.dma_start(out=d_it01_sh[t * P:(t + 1) * P, :], in_=cat[:])

            spmm_pass("i1", t_ue[:, :], D, i1_epilogue)
            tc.strict_bb_all_engine_barrier()
            nc.gpsimd.collective_compute(
                "AllGather", ALU.bypass, replica_groups=groups,
                ins=[d_it01_sh.ap().opt()], outs=[d_it01.ap().opt()])
            tc.strict_bb_all_engine_barrier()

            # ---- P2: [u1|u2] fused
            def uf_epilogue(t, psum):
                u0t = pper.tile([P, D], BF, tag="u0t")
                nc.sync.dma_start(out=u0t[:], in_=t_ue_own[t * P:(t + 1) * P, :])
                r = pe.tile([P, D], F32, tag="r01")
                nc.vector.tensor_scalar(out=r[:], in0=u0t[:], scalar1=cfg.res,
                                        scalar2=None, op0=ALU.mult)
                u1t = pe.tile([P, D], BF, tag="u1t")
                u2t = pe.tile([P, D], BF, tag="u2t")
                if psum is not None:
                    nc.vector.tensor_tensor(out=u1t[:], in0=psum[:, :D], in1=r[:], op=ALU.add)
                    nc.vector.tensor_tensor(out=u2t[:], in0=psum[:, D:], in1=r[:], op=ALU.add)
                else:
                    nc.vector.tensor_copy(out=u1t[:], in_=r[:])
                    nc.vector.tensor_copy(out=u2t[:], in_=r[:])
                nc.sync.dma_start(out=d_u1_sh[t * P:(t + 1) * P, :], in_=u1t[:])
                nc.sync.dma_start(out=d_u2_sh[t * P:(t + 1) * P, :], in_=u2t[:])

            spmm_pass("uf", d_it01[:, :], 2 * D, uf_epilogue)
            tc.strict_bb_all_engine_barrier()
            nc.gpsimd.collective_compute(
                "AllGather", ALU.bypass, replica_groups=groups,
                ins=[d_u1_sh.ap().opt()], outs=[d_u1.ap().opt()])
            tc.strict_bb_all_engine_barrier()

            # ---- P3: i2 (pruned)
            def i2_epilogue(t, psum):
                if psum is None:
                    return
                i0t = pper.tile([P, D], BF, tag="i0t")
                nc.sync.dma_start(out=i0t[:], in_=t_ie_own[t * P:(t + 1) * P, :])
                r = pe.tile([P, D], F32, tag="r01")
                nc.vector.tensor_scalar(out=r[:], in0=i0t[:], scalar1=cfg.res,
                                        scalar2=None, op0=ALU.mult)
                i2t = pe.tile([P, D], BF, tag="i2t")
                nc.vector.tensor_tensor(out=i2t[:], in0=psum[:], in1=r[:], op=ALU.add)
                nc.sync.dma_start(out=d_i2_sh[t * P:(t + 1) * P, :], in_=i2t[:])

            spmm_pass("i2", d_u1[:, :], D, i2_epilogue)

            # ---- P4: y0 = u2 @ Wsoc0
            for t in range(cfg.UT):
                u2t = pper.tile([P, D], BF, tag="u2ld")
                nc.sync.dma_start(out=u2t[:], in_=d_u2_sh[t * P:(t + 1) * P, :])
                psY = xw_rows(u2t, wsoc0_t, D)
                y0t = pe.tile([P, D], BF, tag="y0t")
                nc.vector.tensor_copy(out=y0t[:], in_=psY[:])
                nc.sync.dma_start(out=d_y0_sh[t * P:(t + 1) * P, :], in_=y0t[:])
            tc.strict_bb_all_engine_barrier()
            nc.gpsimd.collective_compute(
                "AllGather", ALU.bypass, replica_groups=groups,
                ins=[d_y0_sh.ap().opt()], outs=[d_y0.ap().opt()])
            tc.strict_bb_all_engine_barrier()

            # ---- P5/P6: social layers
            def social_epilogue(t, psum, cur_sh, out_sh, y_next_w, y_next_sh):
                if psum is None:
                    return
                curt = pper.tile([P, D], BF, tag="curt")
                nc.sync.dma_start(out=curt[:], in_=cur_sh[t * P:(t + 1) * P, :])
                newf = pe.tile([P, D], F32, tag="newf")
                nc.vector.tensor_tensor(out=newf[:], in0=psum[:], in1=curt[:], op=ALU.add)
                newbf = pe.tile([P, D], BF, tag="newbf")
                nc.vector.tensor_copy(out=newbf[:], in_=newf[:])
                psY = xw_rows(newbf, how_t, D)
                t0f = pe.tile([P, D], F32, tag="t0f")
                nc.vector.tensor_tensor(out=t0f[:], in0=psY[:], in1=hob_t[:], op=ALU.add)
                gpre = layernorm_lrelu(t0f, hog_t, hobeta_t)
                gate = pe.tile([P, D], F32, tag="gate")
                nc.scalar.activation(out=gate[:], in_=gpre[:], func=AF.Sigmoid)
                dlt = pe.tile([P, D], F32, tag="dlt")
                nc.vector.tensor_tensor(out=dlt[:], in0=newf[:], in1=curt[:], op=ALU.subtract)
                gd = pe.tile([P, D], F32, tag="gd")
                nc.vector.tensor_tensor(out=gd[:], in0=gate[:], in1=dlt[:], op=ALU.mult)
                curn = pe.tile([P, D], BF, tag="curn")
                nc.vector.tensor_tensor(out=curn[:], in0=curt[:], in1=gd[:], op=ALU.add)
                nc.sync.dma_start(out=out_sh[t * P:(t + 1) * P, :], in_=curn[:])
                if y_next_w is not None:
                    psY2 = xw_rows(curn, y_next_w, D)
                    ynt = pe.tile([P, D], BF, tag="ynt")
                    nc.vector.tensor_copy(out=ynt[:], in_=psY2[:])
                    nc.sync.dma_start(out=y_next_sh[t * P:(t + 1) * P, :], in_=ynt[:])

            spmm_pass("s1", d_y0[:, :], D,
                      lambda t, ps: social_epilogue(t, ps, d_u2_sh, d_s1_sh,
                                                    wsoc1_t, d_y1_sh))
            tc.strict_bb_all_engine_barrier()
            nc.gpsimd.collective_compute(
                "AllGather", ALU.bypass, replica_groups=groups,
                ins=[d_y1_sh.ap().opt()], outs=[d_y1.ap().opt()])
            tc.strict_bb_all_engine_barrier()

            spmm_pass("s2", d_y1[:, :], D,
                      lambda t, ps: social_epilogue(t, ps, d_s1_sh, d_s2_sh,
                                                    None, None))
            tc.strict_bb_all_engine_barrier()

            # ---- P7: batch phase
            def bgather(table_view, idx_t, col, W=D):
                g = pg.tile([P, W], BF, tag="bg")
                nc.gpsimd.indirect_dma_start(
                    out=g[:], out_offset=None, in_=table_view,
                    in_offset=bass.IndirectOffsetOnAxis(ap=idx_t[:, col:col + 1], axis=0))
                return g

            def wsum3(dst_ap, g0, g1, g2):
                a = pe.tile([P, D], F32, tag="ws_a")
                nc.vector.tensor_scalar(out=dst_ap, in0=g0[:], scalar1=float(cfg.nw[0]),
                                        scalar2=None, op0=ALU.mult)
                nc.vector.tensor_scalar(out=a[:], in0=g1[:], scalar1=float(cfg.nw[1]),
                                        scalar2=None, op0=ALU.mult)
                nc.vector.tensor_tensor(out=dst_ap, in0=dst_ap, in1=a[:], op=ALU.add)
                b = pe.tile([P, D], F32, tag="ws_b")
                nc.vector.tensor_scalar(out=b[:], in0=g2[:], scalar1=float(cfg.nw[2]),
                                        scalar2=None, op0=ALU.mult)
                nc.vector.tensor_tensor(out=dst_ap, in0=dst_ap, in1=b[:], op=ALU.add)

            for t in range(cfg.BT):
                gu0 = bgather(t_ue[:, :], pu_g_t, t)
                gu1 = bgather(d_u1[:, :], pu_g_t, t)
                gu2 = bgather(d_u2_sh[:, :], pu_l_t, t)
                gs1 = bgather(d_s1_sh[:, :], pu_l_t, t)
                gs2 = bgather(d_s2_sh[:, :], pu_l_t, t)
                fi = pe.tile([P, 2 * D], F32, tag="fi")
                wsum3(fi[:, :D], gu0, gu1, gu2)
                wsum3(fi[:, D:], gu2, gs1, gs2)
                fibf = pe.tile([P, 2 * D], BF, tag="fibf")
                nc.vector.tensor_copy(out=fibf[:], in_=fi[:])
                psT = ppt.tile([P, P], BF, tag="tr")
                nc.tensor.transpose(out=psT[:], in_=fibf[:], identity=ident_t[:])
                fiT = pe.tile([P, P], BF, tag="xT")
                nc.vector.tensor_copy(out=fiT[:], in_=psT[:])
                psF = ppm.tile([P, D], F32, tag="mm")
                nc.tensor.matmul(out=psF[:], lhsT=fiT[:], rhs=mpw_t[:], start=True, stop=True)
                t0f = pe.tile([P, D], F32, tag="t0f")
                nc.vector.tensor_tensor(out=t0f[:], in0=psF[:], in1=mpb_t[:], op=ALU.add)
                fu = layernorm_lrelu(t0f, mpg_t, mpbeta_t)
                pt = pe.tile([P, D], F32, tag="pt")
                nc.vector.tensor_scalar(out=pt[:], in0=fu[:], scalar1=pu_m_t[:, t:t + 1],
                                        scalar2=None, op0=ALU.mult)
                nc.sync.dma_start(out=d_p_part[t * P:(t + 1) * P, :], in_=pt[:])

                gi01b = bgather(d_it01[:, :], pi_g_t, t, W=2 * D)
                gi2 = bgather(d_i2_sh[:, :], pi_l_t, t)
                qt = pe.tile([P, D], F32, tag="qt")
                wsum3(qt[:], gi01b[:, :D], gi01b[:, D:], gi2)
                qm = pe.tile([P, D], F32, tag="qm")
                nc.vector.tensor_scalar(out=qm[:], in0=qt[:], scalar1=pi_m_t[:, t:t + 1],
                                        scalar2=None, op0=ALU.mult)
                nc.sync.dma_start(out=d_q_part[t * P:(t + 1) * P, :], in_=qm[:])

            tc.strict_bb_all_engine_barrier()
            nc.gpsimd.collective_compute(
                "AllReduce", ALU.add, replica_groups=groups,
                ins=[d_p_part.ap().opt()], outs=[d_p.ap().opt()])
            nc.gpsimd.collective_compute(
                "AllReduce", ALU.add, replica_groups=groups,
                ins=[d_q_part.ap().opt()], outs=[d_q.ap().opt()])
            tc.strict_bb_all_engine_barrier()

            # ---- P8: scores
            acc = pw.tile([P, cfg.BT], F32, tag="acc")
            for t in range(cfg.BT):
                pf = pe.tile([P, D], F32, tag="pf")
                nc.sync.dma_start(out=pf[:], in_=d_p[t * P:(t + 1) * P, :])
                qf = pe.tile([P, D], F32, tag="qf")
                nc.sync.dma_start(out=qf[:], in_=d_q[t * P:(t + 1) * P, :])
                m = pe.tile([P, D], F32, tag="m")
                nc.vector.tensor_tensor(out=m[:], in0=pf[:], in1=qf[:], op=ALU.mult)
                nc.vector.reduce_sum(out=acc[:, t:t + 1], in_=m[:], axis=AX.X)
            psAT = ppt.tile([cfg.BT, P], F32, tag="trf")
            nc.tensor.transpose(out=psAT[:], in_=acc[:], identity=identf_t[:])
            accT = pw.tile([cfg.BT, P], F32, tag="accT")
            nc.vector.tensor_copy(out=accT[:], in_=psAT[:])
            nc.sync.dma_start(
                out=o_scores[:].rearrange("(t p) -> t p", p=P), in_=accT[:])

    nc.compile()
    return nc


# ============================ host preprocessing ============================

def _softmax3(lw):
    lw = np.asarray(lw, dtype=np.float64)[:3]
    e = np.exp(lw - lw.max())
    return e / e.sum()


def _build_blocks(dst, src, w, n_tiles, per_core, ncore, min_blk=1, keep_dst=None):
    dst = np.asarray(dst, dtype=np.int64)
    src = np.asarray(src, dtype=np.int64)
    w = np.asarray(w, dtype=np.float32)
    if keep_dst is not None:
        m = keep_dst[dst]
        dst, src, w = dst[m], src[m], w[m]
    core = dst // per_core
    tl = (dst % per_core) // P
    do = (dst % per_core) % P

    counts = np.zeros((ncore, n_tiles), dtype=np.int64)
    np.add.at(counts, (core, tl), 1)
    nblk = (counts.max(axis=0) + P - 1) // P
    if min_blk > 0:
        nblk = np.maximum(nblk, min_blk)
    NB = int(nblk.sum())
    if NB == 0:
        z = np.zeros((ncore, P, 1))
        return nblk, z.astype(np.int32), z.astype(np.float32), z.astype(np.float32)
    blk_start = np.concatenate([[0], np.cumsum(nblk)])

    srcT = np.zeros((ncore, P, NB), dtype=np.int32)
    doT = np.zeros((ncore, P, NB), dtype=np.float32)
    wT = np.zeros((ncore, P, NB), dtype=np.float32)

    order = np.lexsort((tl, core))
    core_s, tl_s = core[order], tl[order]
    src_s, w_s, do_s = src[order], w[order], do[order]
    keys = core_s * n_tiles + tl_s
    starts = np.searchsorted(keys, np.arange(ncore * n_tiles))
    ends = np.searchsorted(keys, np.arange(ncore * n_tiles) + 1)
    for c in range(ncore):
        for t in range(n_tiles):
            s, e = starts[c * n_tiles + t], ends[c * n_tiles + t]
            cnt = e - s
            nb = int(nblk[t])
            if nb == 0:
                continue
            b0 = int(blk_start[t])
            nslots = nb * P
            bs = np.zeros(nslots, dtype=np.int32)
            bw = np.zeros(nslots, dtype=np.float32)
            bd = np.zeros(nslots, dtype=np.float32)
            bs[:cnt] = src_s[s:e]
            bw[:cnt] = w_s[s:e]
            bd[:cnt] = do_s[s:e]
            srcT[c, :, b0:b0 + nb] = bs.reshape(nb, P).T
            wT[c, :, b0:b0 + nb] = bw.reshape(nb, P).T
            doT[c, :, b0:b0 + nb] = bd.reshape(nb, P).T
    return nblk, srcT, doT, wT


def _batch_arrays(idx, per_core, ncore, bt):
    idx = np.asarray(idx, dtype=np.int64).reshape(bt, P).T
    g = idx.astype(np.int32)
    l = np.zeros((ncore, P, bt), dtype=np.int32)
    m = np.zeros((ncore, P, bt), dtype=np.float32)
    for c in range(ncore):
        own = (idx >= c * per_core) & (idx < (c + 1) * per_core)
        l[c] = np.where(own, idx - c * per_core, 0).astype(np.int32)
        m[c] = own.astype(np.float32)
    return g, l, m


def preprocess(inputs, U, I, ncore=8, prune=True):
    D = 64
    B = int(np.asarray(inputs["users"]).shape[0])
    U_PER = ((U + ncore * P - 1) // (ncore * P)) * P
    I_PER = ((I + ncore * P - 1) // (ncore * P)) * P
    UT, IT, BT = U_PER // P, I_PER // P, B // P
    U_PAD, I_PAD = U_PER * ncore, I_PER * ncore

    ue = np.zeros((U_PAD, D), dtype=np.float32)
    ue[:U] = np.asarray(inputs["user_embedding"], dtype=np.float32)
    ie = np.zeros((I_PAD, D), dtype=np.float32)
    ie[:I] = np.asarray(inputs["item_embedding"], dtype=np.float32)

    ui_src = np.asarray(inputs["ui_src"], dtype=np.int64)
    ui_dst = np.asarray(inputs["ui_dst"], dtype=np.int64)
    ui_w = np.asarray(inputs["ui_w"], dtype=np.float32)
    s_src = np.asarray(inputs["s_src"], dtype=np.int64)
    s_dst = np.asarray(inputs["s_dst"], dtype=np.int64)
    s_w = np.asarray(inputs["s_w"], dtype=np.float32)
    users = np.asarray(inputs["users"], dtype=np.int64)
    pos_items = np.asarray(inputs["pos_items"], dtype=np.int64)

    if prune:
        pos_keep = np.zeros(I_PAD, dtype=bool)
        pos_keep[pos_items] = True
        batch_u = np.zeros(U_PAD, dtype=bool)
        batch_u[users] = True
        s2_edges = batch_u[s_src]
        s1_keep = batch_u.copy()
        s1_keep[s_dst[s2_edges]] = True
        i2_keep, s2_keep = pos_keep, batch_u
    else:
        i2_keep = s1_keep = s2_keep = None

    nblk_i1, i1_src, i1_do, i1_w = _build_blocks(ui_dst, ui_src, ui_w, IT, I_PER, ncore, 1)
    nblk_uf, uf_src, uf_do, uf_w = _build_blocks(ui_src, ui_dst, ui_w, UT, U_PER, ncore, 1)
    nblk_i2, i2_src, i2_do, i2_w = _build_blocks(ui_dst, ui_src, ui_w, IT, I_PER, ncore,
                                                 0 if prune else 1, i2_keep)
    nblk_s1, s1_src, s1_do, s1_w = _build_blocks(s_src, s_dst, s_w, UT, U_PER, ncore,
                                                 0 if prune else 1, s1_keep)
    nblk_s2, s2_src, s2_do, s2_w = _build_blocks(s_src, s_dst, s_w, UT, U_PER, ncore,
                                                 0 if prune else 1, s2_keep)

    nw = _softmax3(inputs["layer_weights"])
    cfg = Cfg(NCORE=ncore, D=D, U_PER=U_PER, I_PER=I_PER, B=B,
              nw=(float(nw[0]), float(nw[1]), float(nw[2])),
              nblk_i1=list(nblk_i1), nblk_uf=list(nblk_uf), nblk_i2=list(nblk_i2),
              nblk_s1=list(nblk_s1), nblk_s2=list(nblk_s2))

    pu_g, pu_l, pu_m = _batch_arrays(users, U_PER, ncore, BT)
    pi_g, pi_l, pi_m = _batch_arrays(pos_items, I_PER, ncore, BT)

    iota = np.tile(np.arange(P, dtype=np.float32), (P, 1))
    ident = np.eye(P, dtype=np.float32)

    rep = lambda v: np.tile(np.asarray(v, dtype=np.float32)[None, :], (P, 1))
    Wsoc = np.asarray(inputs["Wsoc"], dtype=np.float32)

    common = {
        "ue": ue.astype(BF16),
        "iota": iota.astype(BF16),
        "ident": ident.astype(BF16),
        "identf": ident,
        "wsoc0": Wsoc[0].astype(BF16),
        "wsoc1": Wsoc[1].astype(BF16),
        "how": np.asarray(inputs["ho_W"], dtype=np.float32).astype(BF16),
        "mpw": np.asarray(inputs["mp_W"], dtype=np.float32).astype(BF16),
        "hog": rep(inputs["ho_g"]), "hob": rep(inputs["ho_b"]),
        "hobeta": rep(inputs["ho_beta"]),
        "mpg": rep(inputs["mp_g"]), "mpb": rep(inputs["mp_b"]),
        "mpbeta": rep(inputs["mp_beta"]),
        "pu_g": pu_g, "pi_g": pi_g,
    }
    in_maps = []
    for c in range(ncore):
        m = dict(common)
        m["ue_own"] = ue[c * U_PER:(c + 1) * U_PER].astype(BF16)
        m["ie_own"] = ie[c * I_PER:(c + 1) * I_PER].astype(BF16)
        for nm, (srcT, doT, wT) in (("i1", (i1_src, i1_do, i1_w)),
                                    ("uf", (uf_src, uf_do, uf_w)),
                                    ("i2", (i2_src, i2_do, i2_w)),
                                    ("s1", (s1_src, s1_do, s1_w)),
                                    ("s2", (s2_src, s2_do, s2_w))):
            m[f"eb_{nm}_src"] = srcT[c]
            m[f"eb_{nm}_do"] = doT[c]
            m[f"eb_{nm}_w"] = wT[c]
        m["pu_l"], m["pu_m"] = pu_l[c], pu_m[c]
        m["pi_l"], m["pi_m"] = pi_l[c], pi_m[c]
        in_maps.append(m)
    return cfg, in_maps


# ============================ runner ============================

class _Runner:
    def __init__(self, nc, n_cores):
        import jax
        from jax.sharding import Mesh, PartitionSpec
        from jax.experimental.shard_map import shard_map
        from concourse.bass2jax import (_bass_exec_p, install_neuronx_cc_hook,
                                        partition_id_tensor)
        install_neuronx_cc_hook()
        self.jax = jax
        self.PartitionSpec = PartitionSpec
        self.n_cores = n_cores
        partition_name = nc.partition_id_tensor.name if nc.partition_id_tensor else None
        in_names, out_names, out_avals, zero_outs = [], [], [], []
        for alloc in nc.m.functions[0].allocations:
            if not isinstance(alloc, mybir.MemoryLocationSet):
                continue
            name = alloc.memorylocations[0].name
            if alloc.kind == "ExternalInput":
                if name != partition_name:
                    in_names.append(name)
            elif alloc.kind == "ExternalOutput":
                shape = list(alloc.tensor_shape)
                npdt = np.dtype(mybir.dt.np(alloc.dtype))
                out_avals.append(jax.core.ShapedArray(shape, npdt))
                zero_outs.append(np.zeros(shape, npdt))
                out_names.append(name)
        self.in_names = in_names
        self.out_names = out_names
        self.zero_outs = zero_outs
        n_params, n_outs = len(in_names), len(out_names)
        all_in_names = in_names + out_names
        if partition_name is not None:
            all_in_names.append(partition_name)

        def _body(*args):
            operands = list(args)
            if partition_name is not None:
                operands.append(partition_id_tensor())
            outs = _bass_exec_p.bind(
                *operands,
                out_avals=tuple(out_avals),
                in_names=tuple(all_in_names),
                out_names=tuple(out_names),
                lowering_input_output_aliases=(),
                sim_require_finite=True,
                sim_require_nnan=True,
                nc=nc,
            )
            return tuple(outs)

        donate = tuple(range(n_params, n_params + n_outs))
        devices = jax.devices()[:n_cores]
        self.mesh = Mesh(np.asarray(devices), ("core",))
        in_specs = (PartitionSpec("core"),) * (n_params + n_outs)
        out_specs = (PartitionSpec("core"),) * n_outs
        self._fn = jax.jit(
            shard_map(_body, mesh=self.mesh, in_specs=in_specs,
                      out_specs=out_specs, check_rep=False),
            donate_argnums=donate, keep_unused=True)

    def run(self, in_maps):
        jax = self.jax
        sharding = jax.sharding.NamedSharding(self.mesh, self.PartitionSpec("core"))
        args = []
        for n in self.in_names:
            concat = np.concatenate([np.asarray(m[n]) for m in in_maps], axis=0)
            args.append(jax.device_put(concat, sharding))
        zo = [jax.device_put(np.concatenate([z] * self.n_cores, axis=0), sharding)
              for z in self.zero_outs]
        outs = self._fn(*args, *zo)
        jax.block_until_ready(outs)
        res = []
        for c in range(self.n_cores):
            d = {}
            for i, n in enumerate(self.out_names):
                a = np.asarray(outs[i])
                per = a.shape[0] // self.n_cores
                d[n] = a[c * per:(c + 1) * per]
            res.append(d)
        return res


# ============================ entry point ============================

def _kernel_impl(inputs) -> np.ndarray:
    cfg, in_maps = preprocess(inputs, U_FULL, I_FULL, ncore=8, prune=True)
    nc = build(cfg)
    r = _Runner(nc, cfg.NCORE)
    res = r.run(in_maps)
    return np.asarray(res[0]["scores"], dtype=np.float32)


def _kernel_subprocess(inputs) -> np.ndarray:
    """Retry path: run in a fresh process (fresh PJRT client) after a
    transient device failure (NRT_EXEC_UNIT_UNRECOVERABLE wedges recover
    across processes on this fleet)."""
    import os, subprocess, tempfile
    with tempfile.TemporaryDirectory() as td:
        inp_path = os.path.join(td, "in.npz")
        out_path = os.path.join(td, "out.npy")
        np.savez(inp_path, **{k: np.asarray(v) for k, v in inputs.items()})
        code = (
            "import numpy as np, importlib.util\n"
            f"spec = importlib.util.spec_from_file_location('gnnk', {__file__!r})\n"
            "m = importlib.util.module_from_spec(spec); spec.loader.exec_module(m)\n"
            f"d = dict(np.load({inp_path!r}))\n"
            f"np.save({out_path!r}, m._kernel_impl(d))\n"
        )
        env = dict(os.environ)
        env["GNN_KERNEL_SUBPROC"] = "1"
        subprocess.run([sys.executable, "-c", code], check=True, env=env, timeout=1800)
        return np.load(out_path)


def kernel(**inputs) -> np.ndarray:
    import os, time as _time
    if os.environ.get("GNN_KERNEL_SUBPROC"):
        return _kernel_impl(inputs)
    try:
        return _kernel_impl(inputs)
    except Exception:
        _time.sleep(10)
        return _kernel_subprocess(inputs)
